# revision 50
# baseline (speedup 1.0000x reference)
"""Multi-head attention Trainium2 kernel (8 NeuronCores, SPMD).

Problem: nn_MultiHeadAttention (B=2, S=2048, D=768, H=12, d_k=64), f32 I/O.

Sharding: 24 (batch, head) pairs -> 8 cores x 3 heads. Core c handles
batch b = c // 4 and heads [3*(c%4), 3*(c%4)+3). Each core computes the
Q/K/V projections for its 3 heads, full-sequence attention, and its
partial contribution to the output projection. A 4-core ReduceScatter
(cores of the same batch) sums the partials and leaves each core with a
distinct 512-row slice of the batch output; the host concatenates.

On-device layouts are transposed (feature-major) so every matmul maps
directly onto the PE array (out = lhsT.T @ rhs, contraction on the
partition dim):
  - q/k/v are shipped as qT/kT/vT [768, S] bf16 (host transpose + cast)
  - weights shipped pre-transposed/sliced; softmax runs on transposed
    scores sT[kv, q] = K_h Q_h^T so attn @ V becomes V^T @ eT with
    natural-layout V as the stationary operand
  - softmax skips max-subtraction (scores are O(1) for this problem) and
    gets the denominator for free from a ones-column appended to V
  - output bias enters via a ones-row appended to the outT stack
"""

import numpy as np
import ml_dtypes

B = 2
S = 2048
D = 768
H = 12
DK = 64
HPC = 3           # heads per core
HD = HPC * DK     # 192 head-feature columns per core
NCORES = 8
GROUP = 4         # cores per batch (reduce-scatter group)
QS = S // GROUP   # 512 output rows per core

_compiled = None


def _build(reps=1, collective=True):
    """Build the SPMD program. reps>1 emits the whole pipeline N times
    back-to-back (same inputs/outputs) — used only for timing, where
    (T_reps - T_1)/(reps-1) cancels the per-dispatch overhead.
    collective=False drops the final ReduceScatter (for TimelineSim)."""
    import concourse.mybir as mybir
    import concourse.tile as tile
    from concourse import bacc
    from concourse.bass import ts

    bf16 = mybir.dt.bfloat16
    f32 = mybir.dt.float32

    nc = bacc.Bacc(num_devices=NCORES)

    qt = nc.dram_tensor("qt", [D, S], bf16, kind="ExternalInput")
    kt = nc.dram_tensor("kt", [D, S], bf16, kind="ExternalInput")
    vt = nc.dram_tensor("vt", [D, S], bf16, kind="ExternalInput")
    wq = nc.dram_tensor("wq", [D, HD], bf16, kind="ExternalInput")
    wk = nc.dram_tensor("wk", [D, HD], bf16, kind="ExternalInput")
    wv = nc.dram_tensor("wv", [D, HD], bf16, kind="ExternalInput")
    wo = nc.dram_tensor("wo", [HD + 1, D], bf16, kind="ExternalInput")
    bq = nc.dram_tensor("bq", [HD, 1], f32, kind="ExternalInput")
    bk = nc.dram_tensor("bk", [HD, 1], f32, kind="ExternalInput")
    bv = nc.dram_tensor("bv", [1, HD], f32, kind="ExternalInput")
    out_ext = nc.dram_tensor("out", [QS, D], bf16, kind="ExternalOutput")
    out_part = nc.dram_tensor("out_part", [S, D], bf16)
    out_rs = nc.dram_tensor("out_rs", [QS, D], bf16)
    # DRAM bounce rows for broadcasting softmax reciprocals across
    # partitions (SBUF->SBUF partition-broadcast DMA is not allowed).
    rscratch = nc.dram_tensor("rscratch", [HPC * (S // 512), 512], f32)

    RGROUPS = [list(range(g * GROUP, (g + 1) * GROUP))
               for g in range(NCORES // GROUP)]
    NC_ = D // 128      # 6 contraction chunks for the projections
    NKC = S // 128      # 16 kv chunks
    NQB = S // 512      # 4 q blocks
    VW = DK + 2         # 66-wide per-head V block: 64 dims + ones col + pad

    import contextlib

    with tile.TileContext(nc) as tc:
      with (tc.For_i(0, reps, 1) if reps > 1 else contextlib.nullcontext()):
       with contextlib.ExitStack() as ctx:
        sfx = ""
        consts = ctx.enter_context(tc.tile_pool(name="consts" + sfx, bufs=1))
        acts = ctx.enter_context(tc.tile_pool(name="acts" + sfx, bufs=1))

        # ---- load inputs, in consumption order ----
        # DMA bandwidth is the startup bound, so emit transfers in the
        # order compute consumes them (wq -> qt -> wk -> kt -> wv -> vt
        # -> wo), alternating the two HWDGE queues (SP + ACT).
        import concourse.bass as bass
        dmae = [nc.sync, nc.scalar]
        ins_sb, w_sb, bias_sb = {}, {}, {}

        def load_w(name, t):
            sb = consts.tile([128, NC_, HD], bf16, tag=name)
            nc.scalar.dma_start(
                out=sb, in_=t[:, :].rearrange("(c p) n -> p c n", p=128))
            w_sb[name] = sb

        def load_bias(name, t):
            b0 = consts.tile([128, 1], f32, tag=name + "0")
            nc.sync.dma_start(out=b0, in_=t[0:128, :])
            b1 = consts.tile([HD - 128, 1], f32, tag=name + "1")
            nc.sync.dma_start(out=b1, in_=t[128:HD, :])
            bias_sb[name] = (b0, b1)

        def load_in(name, t, di=[0]):
            sb = consts.tile([128, NC_, S], bf16, tag=name)
            for c in range(NC_):
                dmae[di[0] % 2].dma_start(
                    out=sb[:, c, :], in_=t[c * 128:(c + 1) * 128, :])
                di[0] += 1
            ins_sb[name] = sb

        load_w("wk", wk)
        load_bias("bk", bk)
        load_in("kt", kt)
        load_w("wq", wq)
        load_bias("bq", bq)
        load_in("qt", qt)
        load_w("wv", wv)
        bv_bc = consts.tile([128, HD], f32, tag="bv")
        nc.sync.dma_start(
            out=bv_bc,
            in_=bass.AP(tensor=bv[:, :].tensor, offset=bv[:, :].offset,
                        ap=[[0, 128]] + bv[:, :].ap[1:]))
        load_in("vt", vt)
        wo0 = consts.tile([128, D], bf16, tag="wo0")
        nc.scalar.dma_start(out=wo0, in_=wo[0:128, :])
        wo1 = consts.tile([HD + 1 - 128, D], bf16, tag="wo1")
        nc.scalar.dma_start(out=wo1, in_=wo[128:HD + 1, :])
        # Touch the exp table early so ACT's table DMA overlaps the loads.
        warm = consts.tile([1, 1], f32, tag="warm")
        nc.vector.memset(warm, 0.0)
        nc.scalar.activation(out=warm, in_=warm,
                             func=mybir.ActivationFunctionType.Exp)

        # ---- Q/K projections into transposed per-head-group layout ----
        # group 0: heads 0,1 stacked on partitions 0..127; group 1: head 2.
        # The projection accumulators share the scores PSUM pool (same
        # tag) so the attention phase isn't gated on a pool-close
        # boundary: sc 2x3 banks + pv 2x1 = all 8 PSUM banks, one pool
        # lifetime across both phases.
        GRPS = [(0, 128), (128, 64)]
        proj = {}
        # PSUM budget: scores 2x3 banks (exclusively theirs, so attention
        # never waits on projection slot rotation) + 2x1-bank accumulators
        # shared in time by Q/K/V projections and PV = 8 banks.
        with tc.tile_pool(name="sc_psum" + sfx, bufs=2, space="PSUM") as sc_psum, \
                tc.tile_pool(name="acc_psum" + sfx, bufs=4, space="PSUM") as acc_psum:
            def emit_qk_proj(gi):
                off, m = GRPS[gi]
                for name in ("q", "k"):
                    dest = acts.tile([m, S], bf16, tag=f"{name}T{gi}")
                    proj[(name, gi)] = dest
                if gi == 0:
                    for name, wname, bname in (("k", "wk", "bk"),
                                               ("q", "wq", "bq")):
                        x_sb = ins_sb[name + "t"]
                        dest = proj[(name, gi)]
                        bias_ap = bias_sb[bname][gi]
                        for qb in range(NQB):
                            ps = acc_psum.tile([128, 512], f32, tag="acc")
                            for c in range(NC_):
                                nc.tensor.matmul(
                                    ps[0:m, :],
                                    lhsT=w_sb[wname][:, c, off:off + m],
                                    rhs=x_sb[:, c, ts(qb, 512)],
                                    start=(c == 0), stop=(c == NC_ - 1))
                            nc.vector.tensor_scalar_add(
                                out=dest[:, ts(qb, 512)], in0=ps[0:m, :],
                                scalar1=bias_ap[0:m, :])
                else:
                    # M=64 pair: Q-g1 on PE column-groups 0-1, K-g1 on 2-3,
                    # running concurrently in one accumulator tile.
                    for qb in range(NQB):
                        ps = acc_psum.tile([128, 512], f32, tag="acc")
                        for c in range(NC_):
                            nc.tensor.matmul(
                                ps[0:64, :],
                                lhsT=w_sb["wq"][:, c, off:off + 64],
                                rhs=ins_sb["qt"][:, c, ts(qb, 512)],
                                start=(c == 0), stop=(c == NC_ - 1),
                                tile_position=(0, 0))
                            nc.tensor.matmul(
                                ps[64:128, :],
                                lhsT=w_sb["wk"][:, c, off:off + 64],
                                rhs=ins_sb["kt"][:, c, ts(qb, 512)],
                                start=(c == 0), stop=(c == NC_ - 1),
                                tile_position=(0, 64))
                        nc.vector.tensor_scalar_add(
                            out=proj[("q", 1)][:, ts(qb, 512)],
                            in0=ps[0:64, :], scalar1=bias_sb["bq"][1])
                        nc.vector.tensor_scalar_add(
                            out=proj[("k", 1)][:, ts(qb, 512)],
                            in0=ps[64:128, :], scalar1=bias_sb["bk"][1])

            emit_qk_proj(0)

            # ---- attention helpers (emitted piecewise so PE/ACT have
            # scores work while vt still streams in) ----
            outT0 = acts.tile([128, S], bf16, tag="outT0")
            outT1 = acts.tile([DK + 1, S], bf16, tag="outT1")
            nc.vector.memset(outT1[DK:DK + 1, :], 1.0)
            v_sb = acts.tile([128, NKC, HPC * VW], bf16, tag="v")
            for h in range(HPC):
                nc.vector.memset(v_sb[:, :, h * VW + DK:h * VW + DK + 1], 1.0)
            # kc rounds of 3 (+1 tail): scores psum [128,3,512] double-buffered
            # rounds of 2 kc: scores tiles shrink to [128,2,512] so the
            # PSUM budget fits a 4-deep PV accumulator rotation (the
            # norm's DRAM-bounce latency no longer stalls the next PV)
            ROUNDS = [(k0, 2) for k0 in range(0, NKC, 2)]
            sm_pool = ctx.enter_context(tc.tile_pool(name="sm" + sfx, bufs=3))
            nrm_pool = ctx.enter_context(tc.tile_pool(name="nrm" + sfx, bufs=6))

            def head_slices(h):
                if h < 2:
                    return (proj[("q", 0)][ts(h, 64), :],
                            proj[("k", 0)][ts(h, 64), :])
                return (proj[("q", 1)][0:64, :], proj[("k", 1)][0:64, :])

            def emit_scores(h, qb):
                qth, kth = head_slices(h)
                expt = sm_pool.tile([128, NKC, 512], bf16, tag="expt")
                for k0, klen in ROUNDS:
                    scps = sc_psum.tile([128, 2, 512], f32, tag="sc")
                    for j in range(klen):
                        nc.tensor.matmul(
                            scps[:, j, :],
                            lhsT=kth[:, ts(k0 + j, 128)],
                            rhs=qth[:, ts(qb, 512)],
                            start=True, stop=True)
                    nc.scalar.activation(
                        out=expt[:, k0:k0 + klen, :],
                        in_=scps[:, 0:klen, :],
                        func=mybir.ActivationFunctionType.Exp,
                        scale=float(1.0 / np.sqrt(DK)))
                return expt

            def emit_pv_norm(h, qb, expt):
                pvps = acc_psum.tile([DK + 1, 512], f32, tag="acc")
                for kc in range(NKC):
                    nc.tensor.matmul(
                        pvps,
                        lhsT=v_sb[:, kc, h * VW:h * VW + DK + 1],
                        rhs=expt[:, kc, :],
                        start=(kc == 0), stop=(kc == NKC - 1))
                recip = nrm_pool.tile([1, 512], f32, tag="recip")
                nc.vector.reciprocal(recip, pvps[DK:DK + 1, :])
                row = rscratch[h * NQB + qb:h * NQB + qb + 1, :]
                nc.sync.dma_start(out=row, in_=recip)
                rbc = nrm_pool.tile([64, 512], f32, tag="rbc")
                nc.sync.dma_start(
                    out=rbc,
                    in_=bass.AP(tensor=row.tensor, offset=row.offset,
                                ap=[[0, 64]] + row.ap[1:]))
                dst = (outT0[ts(h, 64), ts(qb, 512)] if h < 2
                       else outT1[0:64, ts(qb, 512)])
                nc.vector.tensor_mul(dst, pvps[0:DK, :], rbc)

            # scores for (h0, qb0..2) ahead of the V projection so the
            # exp stream never starves while vt loads / projects
            early = [emit_scores(0, 0), emit_scores(0, 1)]
            emit_qk_proj(1)
            e02 = emit_scores(0, 2)

            # ---- V projection in natural layout, 66-stride head blocks ----
            for st in range(NKC):
                ps = acc_psum.tile([128, 512], f32, tag="acc")
                for c in range(NC_):
                    nc.tensor.matmul(
                        ps[:, 0:HD],
                        lhsT=ins_sb["vt"][:, c, ts(st, 128)],
                        rhs=w_sb["wv"][:, c, :],
                        start=(c == 0), stop=(c == NC_ - 1))
                for h in range(HPC):
                    nc.vector.tensor_add(
                        v_sb[:, st, h * VW:h * VW + DK],
                        ps[:, ts(h, 64)], bv_bc[:, ts(h, 64)])

            # software-pipelined: emit scores(step i) right after
            # pv(step i-3), so ACT always has the next block's scores
            # matmuls queued while PV / norm chains drain
            SEQ = [(h, qb) for h in range(HPC) for qb in range(NQB)]
            pend = {(0, 0): early[0], (0, 1): early[1], (0, 2): e02}
            for i, hq in enumerate(SEQ):
                if i >= 3:
                    prev = SEQ[i - 3]
                    emit_pv_norm(*prev, pend.pop(prev))
                if hq not in pend:
                    pend[hq] = emit_scores(*hq)
            for hq in SEQ[-3:]:
                emit_pv_norm(*hq, pend.pop(hq))

        # ---- output projection (bias via outT1 ones row x wo row 192) ----
        with tc.tile_pool(name="fo_psum" + sfx, bufs=2, space="PSUM") as fo_psum, \
                tc.tile_pool(name="fo" + sfx, bufs=2) as fo_pool:
            for og in range(4):            # 4 output groups of 4 q-tiles
                ot = fo_pool.tile([128, 4, D], bf16, tag="ot")
                for sq in range(4):
                    qt_ = og * 4 + sq
                    ps = fo_psum.tile([128, D], f32, tag="fo")
                    for noff, nsz in ((0, 512), (512, 256)):
                        nc.tensor.matmul(
                            ps[:, noff:noff + nsz],
                            lhsT=outT0[:, ts(qt_, 128)],
                            rhs=wo0[:, noff:noff + nsz],
                            start=True, stop=False)
                        nc.tensor.matmul(
                            ps[:, noff:noff + nsz],
                            lhsT=outT1[:, ts(qt_, 128)],
                            rhs=wo1[:, noff:noff + nsz],
                            start=False, stop=True)
                    nc.vector.tensor_copy(out=ot[:, sq, :], in_=ps)
                nc.sync.dma_start(
                    out=out_part[:, :].rearrange(
                        "(g t p) d -> g p t d", p=128, t=4)[og],
                    in_=ot)
                if collective:
                    nc.gpsimd.collective_compute(
                        "ReduceScatter", mybir.AluOpType.add,
                        replica_groups=RGROUPS,
                        ins=[out_part[ts(og, 512), :]],
                        outs=[out_rs[ts(og, 128), :]])
                nc.sync.dma_start(out=out_ext[ts(og, 128), :],
                                  in_=(out_rs if collective
                                       else out_part)[ts(og, 128), :])


    nc.compile()
    return nc


def _get_compiled():
    global _compiled
    if _compiled is None:
        _compiled = _build()
    return _compiled


def make_in_maps(q, k, v, Wq, bq, Wk, bk, Wv, bv, Wo, bo):
    bf = ml_dtypes.bfloat16
    in_maps = []
    for c in range(NCORES):
        b = c // GROUP
        g = c % GROUP
        cols = slice(g * HD, (g + 1) * HD)   # head-feature columns
        wo_aug = np.empty((HD + 1, D), np.float32)
        wo_aug[:HD] = Wo.T[cols.start:cols.stop, :]
        wo_aug[HD] = bo / GROUP              # summed GROUP times by the RS
        in_maps.append({
            "qt": np.ascontiguousarray(q[b].T).astype(bf),
            "kt": np.ascontiguousarray(k[b].T).astype(bf),
            "vt": np.ascontiguousarray(v[b].T).astype(bf),
            "wq": np.ascontiguousarray(Wq.T[:, cols]).astype(bf),
            "wk": np.ascontiguousarray(Wk.T[:, cols]).astype(bf),
            "wv": np.ascontiguousarray(Wv.T[:, cols]).astype(bf),
            "wo": wo_aug.astype(bf),
            "bq": np.ascontiguousarray(bq[cols].reshape(HD, 1)).astype(np.float32),
            "bk": np.ascontiguousarray(bk[cols].reshape(HD, 1)).astype(np.float32),
            "bv": np.ascontiguousarray(bv[cols].reshape(1, HD)).astype(np.float32),
        })
    return in_maps


def kernel(q, k, v, Wq, bq, Wk, bk, Wv, bv, Wo, bo):
    from concourse.bass_utils import run_bass_kernel_spmd

    q = np.asarray(q, np.float32)
    k = np.asarray(k, np.float32)
    v = np.asarray(v, np.float32)
    nc = _get_compiled()
    in_maps = make_in_maps(q, k, v,
                           np.asarray(Wq, np.float32), np.asarray(bq, np.float32),
                           np.asarray(Wk, np.float32), np.asarray(bk, np.float32),
                           np.asarray(Wv, np.float32), np.asarray(bv, np.float32),
                           np.asarray(Wo, np.float32), np.asarray(bo, np.float32))
    res = run_bass_kernel_spmd(nc, in_maps, list(range(NCORES))).results
    out = np.empty((B, S, D), np.float32)
    for c in range(NCORES):
        b = c // GROUP
        j = c % GROUP
        # chunked reduce-scatter: chunk g of core (b, j) holds batch-b
        # rows [512*g + 128*j, 512*g + 128*j + 128)
        chunks = res[c]["out"].reshape(GROUP, 128, D)
        for g in range(GROUP):
            out[b, 512 * g + 128 * j:512 * g + 128 * j + 128, :] = chunks[g]
    return out

